# revision 5
# baseline (speedup 1.0000x reference)
"""Neighbor-list kernel for trn2 (8 NeuronCores, SPMD-by-row-blocks).

Strategy
--------
all_pairs is verified (host-side) to be the strict-upper-triangle enumeration
of an N=8192 atom set, so the kernel never reads or gathers it: each core
recomputes its contiguous pair-slab directly from coords (96 KB) and streams
out masked (i,j)/(-1,-1) pairs.

Per core (row-blocks of 128 atoms on SBUF partitions, j on the free dim):
  dx   = xj_bcast - xi          (DVE tensor_scalar, per-partition scalar)
  a    = |dx|                   (ACT Abs)
  q,r  = a^2, (a-L)^2           (ACT Square, bias=-L)   -> min-image component
  d2   = min(q,r); s = d2x+d2y+d2z                       (DVE)
  m    = s < T*                 (T* = exact fp32 threshold equal to
                                 sqrt(s) < cutoff under round-to-nearest)
  col0 = m*(i+1)-1, col1 = m*(j+1)-1 -> int32, interleaved
  indirect-DMA scatter of each 128-row x W tile to its (quadratic) pair-row
  offsets in the output slab.

The j-side row broadcasts are built with an exact PE ones-matmul (1.0*x is
bitwise exact) once per 2048-wide window. The 128-wide diagonal wedge of each
row-block (1.5% of pairs) plus the final partial block are computed on the
host with bit-identical fp32 numpy and pasted into the assembled output.
"""

import threading
import numpy as np

N = 8192
G = 2048          # j-window width
NBLK = N // 128   # 64
NCORES = 8

_cache = {}


def _o(i):
    # number of pairs before row i in the triu enumeration
    return i * N - (i * (i + 1)) // 2


def _exact_threshold(cutoff):
    """Smallest float32 t such that sqrt_f32(t) >= cutoff; mask (s < t) then
    matches (sqrt_f32(s) < cutoff) exactly."""
    c = np.float32(cutoff)
    t = np.float32(c * c)
    # move down while sqrt still >= c
    while np.float32(np.sqrt(np.float32(np.nextafter(t, np.float32(0.0))))) >= c:
        t = np.float32(np.nextafter(t, np.float32(0.0)))
    # move up if sqrt(t) < c
    while np.float32(np.sqrt(t)) < c:
        t = np.float32(np.nextafter(t, np.float32(np.inf)))
    return float(t)


def _host_pair_mask(xs, ys, zs, box, tstar, i_idx, j_idx):
    """Bit-identical fp32 min-image mask for given (i, j) index arrays."""
    s = np.zeros(len(i_idx), np.float32)
    for arr, L in ((xs, box[0]), (ys, box[1]), (zs, box[2])):
        d = arr[j_idx] - arr[i_idx]          # f32 sub
        a = np.abs(d)
        q = a * a
        r = (a - np.float32(L)) * (a - np.float32(L))
        s += np.minimum(q, r)
    return s < np.float32(tstar)


def _plan_cores():
    """Assign contiguous row-blocks to cores, balancing device pair counts."""
    dp = [128 * (N - 128 - 128 * b) if b < NBLK - 1 else 0 for b in range(NBLK)]
    total = sum(dp)
    bounds = [0]
    acc = 0
    target = total / NCORES
    for b in range(NBLK):
        acc += dp[b]
        if acc >= target * len(bounds) and len(bounds) < NCORES:
            bounds.append(b + 1)
    while len(bounds) < NCORES:
        bounds.append(NBLK)
    bounds.append(NBLK)
    cores = []
    for c in range(NCORES):
        b0, b1 = bounds[c], bounds[c + 1]
        if c == NCORES - 1:
            b1 = NBLK
        cores.append((b0, b1))
    return cores


def _tiles_for_core(b0, b1):
    """(block, jstart, width, cell) tiles, grouped by window cell."""
    tiles = []
    for b in range(b0, min(b1, NBLK - 1)):
        i0 = 128 * b
        jdev = i0 + 128
        c0 = jdev // G
        for cell in range(c0, N // G):
            jstart = max(G * cell, jdev)
            w = G * (cell + 1) - jstart
            if w > 0:
                tiles.append((b, jstart, w, cell))
    tiles.sort(key=lambda t: (t[3], t[0]))
    return tiles


def _build_core_program(core_meta):
    import concourse.bacc as bacc
    import concourse.tile as tile
    import concourse.mybir as mybir
    from concourse.bass import IndirectOffsetOnAxis

    dt = mybir.dt
    Alu = mybir.AluOpType
    Act = mybir.ActivationFunctionType

    tiles = core_meta["tiles"]
    nblocks = core_meta["nblocks"]
    slab_pairs = core_meta["slab_pairs"]
    box = core_meta["box"]
    tstar = core_meta["tstar"]
    nt = len(tiles)

    nc = bacc.Bacc(None)
    rows4 = nc.declare_dram_parameter("rows4", [4, N], dt.float32, isOutput=False)  # xs|ys|zs|jp1(unused on device)
    cols = nc.declare_dram_parameter("cols", [128, 4 * nblocks], dt.float32, isOutput=False)
    offs = nc.declare_dram_parameter("offs", [128, nt], dt.int32, isOutput=False)
    out = nc.declare_dram_parameter("out", [slab_pairs, 2], dt.int32, isOutput=True)

    with tile.TileContext(nc) as tc:
        with (
            tc.tile_pool(name="inp", bufs=1) as inp,
            tc.tile_pool(name="bc", bufs=1) as bcp,
            tc.tile_pool(name="ps", bufs=2, space="PSUM") as psp,
            tc.tile_pool(name="wk2", bufs=2) as wk2,
            tc.tile_pool(name="wk1", bufs=1) as wk1,
            tc.tile_pool(name="outp", bufs=2) as outp,
        ):
            t_rows = inp.tile([65, N], dt.float32, tag="rows")
            for comp in range(3):
                nc.sync.dma_start(t_rows[32 * comp:32 * comp + 1, :],
                                  rows4[comp:comp + 1, :])
            t_cols = inp.tile([128, 4 * nblocks], dt.float32, tag="cols")
            t_offs = inp.tile([128, nt], dt.int32, tag="offs")
            nc.sync.dma_start(t_cols[:], cols[:])
            nc.sync.dma_start(t_offs[:], offs[:])

            ones = inp.tile([65, 128], dt.float32, tag="ones")
            nc.gpsimd.memset(ones[:], 1.0)
            tbias = inp.tile([128, 3], dt.float32, tag="bias")
            for k in range(3):
                nc.gpsimd.memset(tbias[:, k:k + 1], -float(box[k]))

            cur_cell = -1
            bcast = None
            for (b, jstart, w, cell) in tiles:
                if cell != cur_cell:
                    cur_cell = cell
                    j0 = G * cell
                    # build 3 coord broadcast tiles via exact PE ones-matmul
                    bcast = []
                    for comp in range(3):
                        bp = 32 * comp
                        pt = psp.tile([128, G], dt.float32, tag="ps")
                        for k in range(G // 512):
                            nc.tensor.matmul(
                                pt[:, 512 * k:512 * (k + 1)],
                                ones[bp:bp + 1, :],
                                t_rows[bp:bp + 1, j0 + 512 * k:j0 + 512 * (k + 1)],
                                start=True, stop=True,
                            )
                        bt = bcp.tile([128, G], dt.float32, tag=f"bc{comp}")
                        nc.scalar.copy(bt[:], pt[:])
                        bcast.append(bt)
                    # j+1 broadcast via iota (exact ints) + cast to f32
                    ji = wk1.tile([128, G], dt.int32, tag="q")
                    nc.gpsimd.iota(ji[:], pattern=[[1, G]], base=j0 + 1,
                                   channel_multiplier=0)
                    jb = bcp.tile([128, G], dt.float32, tag="bc3")
                    nc.vector.tensor_copy(jb[:], ji[:])
                    bcast.append(jb)

                cs = jstart - G * cell
                kb = b - core_meta["b0"]
                xi = t_cols[:, 4 * kb + 0:4 * kb + 1]
                yi = t_cols[:, 4 * kb + 1:4 * kb + 2]
                zi = t_cols[:, 4 * kb + 2:4 * kb + 3]
                i1f = t_cols[:, 4 * kb + 3:4 * kb + 4]

                s = wk1.tile([128, G], dt.float32, tag="s")
                for comp, ci in enumerate((xi, yi, zi)):
                    dx = wk2.tile([128, G], dt.float32, tag="dx")
                    nc.vector.tensor_scalar(dx[:, :w], bcast[comp][:, cs:cs + w],
                                            ci, None, Alu.subtract)
                    a = wk2.tile([128, G], dt.float32, tag="a")
                    nc.scalar.activation(a[:, :w], dx[:, :w], Act.Abs)
                    q = wk1.tile([128, G], dt.float32, tag="q")
                    nc.scalar.activation(q[:, :w], a[:, :w], Act.Square)
                    r = wk1.tile([128, G], dt.float32, tag="r")
                    nc.scalar.activation(r[:, :w], a[:, :w], Act.Square,
                                         bias=tbias[:, comp:comp + 1], scale=1.0)
                    if comp == 0:
                        nc.vector.tensor_tensor(s[:, :w], q[:, :w], r[:, :w], Alu.min)
                    else:
                        nc.vector.tensor_tensor(q[:, :w], q[:, :w], r[:, :w], Alu.min)
                        nc.vector.tensor_tensor(s[:, :w], s[:, :w], q[:, :w], Alu.add)

                m = wk1.tile([128, G], dt.float32, tag="m")
                nc.vector.tensor_scalar(m[:, :w], s[:, :w], tstar, None, Alu.is_lt)
                ot = outp.tile([128, 2 * G], dt.int32, tag="ot")
                nc.vector.tensor_scalar(ot[:, 0:2 * w:2], m[:, :w], i1f, -1.0,
                                        Alu.mult, Alu.add)
                c1 = wk1.tile([128, G], dt.float32, tag="r")
                nc.vector.tensor_tensor(c1[:, :w], m[:, :w],
                                        bcast[3][:, cs:cs + w], Alu.mult)
                nc.vector.tensor_scalar(ot[:, 1:2 * w:2], c1[:, :w], -1.0,
                                        None, Alu.add)

                ti = core_meta["tile_index"][(b, jstart)]
                nc.gpsimd.indirect_dma_start(
                    out=out[:],
                    out_offset=IndirectOffsetOnAxis(ap=t_offs[:, ti:ti + 1], axis=0),
                    in_=ot[:, 0:2 * w],
                    in_offset=None,
                )

    nc.finalize()
    return nc


def _get_state(coords, box, cutoff):
    key = ("v1", tuple(np.asarray(box, np.float32).tolist()), float(cutoff))
    data_key = hash(coords.tobytes())
    if key in _cache:
        state = _cache[key]
        if state["data_key"] != data_key:
            _refresh_inputs(state, coords)
            state["data_key"] = data_key
        return state

    import jax
    from concourse import bass2jax
    from concourse.bass2jax import _bass_exec_p, partition_id_tensor
    import concourse.mybir as mybir

    bass2jax.install_neuronx_cc_hook()

    xs = np.ascontiguousarray(coords[:, 0], np.float32)
    ys = np.ascontiguousarray(coords[:, 1], np.float32)
    zs = np.ascontiguousarray(coords[:, 2], np.float32)
    tstar = _exact_threshold(cutoff)
    cores = _plan_cores()

    jp1 = np.arange(1, N + 1, dtype=np.float32)
    rows4 = np.stack([xs, ys, zs, jp1], axis=0)

    metas = []
    for c, (b0, b1) in enumerate(cores):
        tiles = _tiles_for_core(b0, b1)
        nblocks = max(1, min(b1, NBLK - 1) - b0)
        r0, r1 = 128 * b0, 128 * b1
        slab0, slab1 = _o(r0), _o(r1)
        cols = np.zeros((128, 4 * nblocks), np.float32)
        for kb in range(nblocks):
            i0 = 128 * (b0 + kb)
            idx = np.arange(i0, i0 + 128)
            cols[:, 4 * kb + 0] = xs[idx]
            cols[:, 4 * kb + 1] = ys[idx]
            cols[:, 4 * kb + 2] = zs[idx]
            cols[:, 4 * kb + 3] = (idx + 1).astype(np.float32)
        offs = np.zeros((128, max(1, len(tiles))), np.int32)
        tile_index = {}
        for t, (b, jstart, w, cell) in enumerate(tiles):
            i0 = 128 * b
            ii = np.arange(i0, i0 + 128)
            offs[:, t] = ((ii * N - (ii * (ii + 1)) // 2)
                          + jstart - ii - 1 - slab0).astype(np.int32)
            tile_index[(b, jstart)] = t
        metas.append({
            "core": c, "b0": b0, "b1": b1, "tiles": tiles,
            "nblocks": nblocks, "slab_pairs": slab1 - slab0,
            "slab0": slab0, "slab1": slab1,
            "rows4": rows4, "cols": cols, "offs": offs,
            "tile_index": tile_index, "box": [float(b) for b in box],
            "tstar": tstar,
        })

    # Build + finalize the 8 programs (python-side; serial)
    for m in metas:
        m["nc"] = _build_core_program(m)

    # Per-core jitted executables
    devices = jax.devices()[:NCORES]
    for m in metas:
        nc = m["nc"]
        in_names = ["rows4", "cols", "offs"]
        out_names = ["out"]
        out_avals = [jax.core.ShapedArray((m["slab_pairs"], 2), np.int32)]
        all_names = in_names + out_names
        pid_name = nc.partition_id_tensor.name if nc.partition_id_tensor else None
        if pid_name is not None:
            all_names = all_names + [pid_name]
        donate = (len(in_names),)

        def _body(*args, _nc=nc, _avals=tuple(out_avals), _names=tuple(all_names),
                  _onames=tuple(out_names), _pid=(pid_name is not None)):
            operands = list(args)
            if _pid:
                operands.append(partition_id_tensor())
            outs = _bass_exec_p.bind(
                *operands,
                out_avals=_avals,
                in_names=_names,
                out_names=_onames,
                lowering_input_output_aliases=(),
                sim_require_finite=False,
                sim_require_nnan=False,
                nc=_nc,
            )
            return tuple(outs)

        m["jit"] = jax.jit(_body, donate_argnums=donate, keep_unused=True)
        dev = devices[m["core"]]
        m["dev_inputs"] = [
            jax.device_put(m["rows4"], dev),
            jax.device_put(m["cols"], dev),
            jax.device_put(m["offs"], dev),
        ]
        m["dev_zero"] = jax.device_put(np.zeros((m["slab_pairs"], 2), np.int32), dev)

    # Parallel first-call compile (neuronxcc runs as subprocess -> threads OK)
    def _compile(m):
        m["jit_c"] = m["jit"].lower(*m["dev_inputs"], m["dev_zero"]).compile()

    threads = [threading.Thread(target=_compile, args=(m,)) for m in metas]
    for t in threads:
        t.start()
    for t in threads:
        t.join()

    state = {"metas": metas, "tstar": tstar,
             "xs": xs, "ys": ys, "zs": zs, "data_key": data_key}
    _cache[key] = state
    return state


def _refresh_inputs(state, coords):
    import jax
    devices = jax.devices()[:NCORES]
    xs = np.ascontiguousarray(coords[:, 0], np.float32)
    ys = np.ascontiguousarray(coords[:, 1], np.float32)
    zs = np.ascontiguousarray(coords[:, 2], np.float32)
    state["xs"], state["ys"], state["zs"] = xs, ys, zs
    jp1 = np.arange(1, N + 1, dtype=np.float32)
    rows4 = np.stack([xs, ys, zs, jp1], axis=0)
    for m in state["metas"]:
        nblocks = m["nblocks"]
        b0 = m["b0"]
        cols = np.zeros((128, 4 * nblocks), np.float32)
        for kb in range(nblocks):
            i0 = 128 * (b0 + kb)
            idx = np.arange(i0, i0 + 128)
            cols[:, 4 * kb + 0] = xs[idx]
            cols[:, 4 * kb + 1] = ys[idx]
            cols[:, 4 * kb + 2] = zs[idx]
            cols[:, 4 * kb + 3] = (idx + 1).astype(np.float32)
        dev = devices[m["core"]]
        m["dev_inputs"] = [
            jax.device_put(rows4, dev),
            jax.device_put(cols, dev),
            jax.device_put(m["offs"], dev),
        ]


def _dispatch_all(state):
    """Dispatch all 8 core programs asynchronously; returns output arrays."""
    outs = []
    for m in state["metas"]:
        zero = m.get("next_zero", None)
        if zero is None:
            zero = m["dev_zero"]
        (o_,) = m["jit_c"](*m["dev_inputs"], zero)
        m["next_zero"] = o_       # donate this run's output as next scratch
        outs.append(o_)
    for o_ in outs:
        o_.block_until_ready()
    return outs


def _host_wedges(state, box, out_full):
    xs, ys, zs = state["xs"], state["ys"], state["zs"]
    tstar = state["tstar"]
    boxf = [np.float32(b) for b in box]
    for b in range(NBLK):
        i0 = 128 * b
        hi = min(i0 + 128, N)
        ii = []
        jj = []
        for i in range(i0, hi):
            j_end = min(i0 + 128, N)
            if i + 1 < j_end:
                jr = np.arange(i + 1, j_end)
                ii.append(np.full(len(jr), i))
                jj.append(jr)
        if not ii:
            continue
        i_idx = np.concatenate(ii)
        j_idx = np.concatenate(jj)
        mask = _host_pair_mask(xs, ys, zs, boxf, tstar, i_idx, j_idx)
        pos = (i_idx.astype(np.int64) * N - (i_idx.astype(np.int64) *
               (i_idx.astype(np.int64) + 1)) // 2) + j_idx - i_idx - 1
        out_full[pos, 0] = np.where(mask, i_idx, -1).astype(np.int32)
        out_full[pos, 1] = np.where(mask, j_idx, -1).astype(np.int32)
    # last block (i0 = N-128): fully host-computed above via wedge ranges
    return out_full


def _fallback_host(coords, box, cutoff, all_pairs):
    """Pure-numpy fallback (chunked) for non-triu inputs."""
    tstar = _exact_threshold(cutoff)
    P = all_pairs.shape[0]
    out = np.empty((P, 2), np.int32)
    npairs = 0
    cs = 4 << 20
    c = coords.astype(np.float32)
    for s0 in range(0, P, cs):
        sl = slice(s0, min(P, s0 + cs))
        i_idx = all_pairs[sl, 0].astype(np.int64)
        j_idx = all_pairs[sl, 1].astype(np.int64)
        s = np.zeros(len(i_idx), np.float32)
        for k in range(3):
            d = c[j_idx, k] - c[i_idx, k]
            a = np.abs(d)
            q = a * a
            r = (a - np.float32(box[k])) * (a - np.float32(box[k]))
            s += np.minimum(q, r)
        mask = s < np.float32(tstar)
        out[sl, 0] = np.where(mask, i_idx, -1).astype(np.int32)
        out[sl, 1] = np.where(mask, j_idx, -1).astype(np.int32)
        npairs += int(mask.sum())
    return out, np.int32(npairs)


def _is_triu(all_pairs):
    P = all_pairs.shape[0]
    if P != (N * (N - 1)) // 2:
        return False
    # spot check a spread of rows, then full check
    iu0 = all_pairs[:, 0]
    iu1 = all_pairs[:, 1]
    for i in (0, 1, 7, 100, 4095, 8190):
        o = _o(i)
        if iu0[o] != i or iu1[o] != i + 1:
            return False
    i_idx = np.repeat(np.arange(N - 1, dtype=np.int32),
                      np.arange(N - 1, 0, -1, dtype=np.int64))
    if not np.array_equal(iu0, i_idx):
        return False
    j_expected = np.arange(P, dtype=np.int64) - (
        i_idx.astype(np.int64) * N - (i_idx.astype(np.int64) *
        (i_idx.astype(np.int64) + 1)) // 2) + i_idx + 1
    return np.array_equal(iu1, j_expected.astype(np.int32))


def kernel(coords, box, cutoff, all_pairs):
    coords = np.asarray(coords, np.float32)
    box = np.asarray(box, np.float32)
    cutoff = float(np.asarray(cutoff))
    all_pairs = np.asarray(all_pairs, np.int32)

    if coords.shape != (N, 3) or not _is_triu(all_pairs):
        return _fallback_host(coords, box, cutoff, all_pairs)

    state = _get_state(coords, box, cutoff)
    outs = _dispatch_all(state)

    P = (N * (N - 1)) // 2
    out_full = np.empty((P, 2), np.int32)
    for m, o_ in zip(state["metas"], outs):
        out_full[m["slab0"]:m["slab1"]] = np.asarray(o_)
    _host_wedges(state, box, out_full)

    npairs = np.int32(np.count_nonzero(out_full[:, 0] != -1))
    return out_full, npairs


def bench(coords, box, cutoff, all_pairs, iters=5):
    """Measure device wall time of the 8-core dispatch (excludes host work)."""
    import time
    state = _get_state(np.asarray(coords, np.float32),
                       np.asarray(box, np.float32), float(np.asarray(cutoff)))
    _dispatch_all(state)  # warm
    times = []
    for _ in range(iters):
        t0 = time.perf_counter()
        _dispatch_all(state)
        times.append(time.perf_counter() - t0)
    return min(times), sorted(times)[len(times) // 2]
